# revision 42
# baseline (speedup 1.0000x reference)
"""Pairwise IoU (8192x8192) on 8 Trainium2 NeuronCores via Bass/Tile.

Strategy
--------
Boxes can only overlap when both their x- and y-ranges are within the
max box extent of each other, so most of the 8192x8192 IoU matrix is
exactly zero.  Two-level windowing exploits this: boxes1 rows are
x-sorted and split across 8 cores (1024 rows each); within a core, rows
are y-sorted, and the core's x-relevant boxes2 subset (the only columns
that can ever overlap its rows) is Y1-sorted.  Each 128-row i-tile then
only scores a contiguous window of ~1.2-2.6k y-overlapping columns
(~18% of M on average).  Provably-zero entries are assembled on the
host.

The device program is SPMD (one program, 8 cores), so per-tile window
offsets/widths are compile-time constants OFFS[t]/WT[t]; the host packs
each core's columns so that its tile-t window lies inside
[base_c + OFFS[t], base_c + OFFS[t] + WT[t]).  Per-core tile groups are
assigned to slots in width-descending order (so the max-over-cores per
slot is tight), then the (base, OFFS, WT) decomposition is solved as an
LP from the actual data at first call.

Per-core device kernel, per [128, WT] output tile:
  rx    = relu(min(x2_i, X2_j) - max(x1_i, X1_j))   custom DVE op (fp16 out)
  ry    = same for y                                custom DVE op (fp16 out)
  inter = rx*ry                                     DVE tensor_tensor (fp16, 2x)
  u     = a1_i + a2e_j - inter                      TensorE (rank-2 [ones;a1]
                                                    + (-I)@inter, fp16) -> PSUM
  rinv  = Reciprocal(u)                             ScalarE LUT, one pass
  out   = inter * rinv                              DVE tensor_tensor (bf16 out)

Coordinate planes are host-replicated across partitions and streamed in
as per-plane 2-D DMAs over both hardware-DGE rings (Sync + Scalar) in
processing-order chunks; output stores ride the GpSimd software-DGE
queue so they never contend with input descriptor generation.  (An
on-chip partition-broadcast variant was measured slower: the GpSimd Q7
daisy chain floods the shared POOL SBUF port and inflates every
concurrent DVE op by 2-6x.)
"""

import numpy as np

N = 8192
M = 8192
NCORES = 8
ROWS = N // NCORES  # rows of boxes1 per core
P = 128  # partitions
NT = ROWS // P  # 8 i-tiles per core
PS = 512  # psum bank width (fp32)
EPS = 1e-7

_COMPILED = {}


def _register_op(name, spec, subdim=False):
    import concourse.dve_ops as dve_ops
    from concourse.dve_spec import lower
    from concourse.dve_uop import DveOpSpec

    for op in dve_ops.OPS:
        if op.name == name:
            return op
    shas = {}
    for ver in ("v3", "v4"):
        try:
            shas[ver] = DveOpSpec(
                name=name, opcode=0, uops=lower(spec, ver=ver)
            ).sha(ver)
        except Exception:
            pass
    op = dve_ops.DveOp(name, spec, subdim=subdim, uops_sha=shas)
    dve_ops.OPS.append(op)
    dve_ops.CUSTOM_DVE_SPECS[op.name] = op.spec
    dve_ops._SUB_OPCODE_FOR_NAME[op.name] = (
        dve_ops._CUSTOM_DVE_ROW_BASE + len(dve_ops.OPS) - 1
    )
    return op


def _ensure_ops():
    """Register the IOU_EDGE custom DVE op (idempotent)."""
    from concourse.dve_spec import C0, C1, Spec, Src0, Src1, maxx, minn, relu

    edge = _register_op(
        "IOU_EDGE",
        Spec(
            body=relu(minn(Src1, C1) - maxx(Src0, C0)),
            reference=lambda in0, in1, s0, s1, imm2: np.maximum(
                np.minimum(in1, s1) - np.maximum(in0, s0), 0.0
            ).astype(np.float32),
        ),
    )
    return edge


def _build_program(WT, OFFS, WCOL):
    from contextlib import ExitStack

    import concourse.bacc as bacc
    import concourse.mybir as mybir
    import concourse.tile as tile

    iou_edge = _ensure_ops()

    f32 = mybir.dt.float32
    f16 = mybir.dt.float16
    bf16 = mybir.dt.bfloat16
    act = mybir.ActivationFunctionType
    nc = bacc.Bacc(
        "TRN2",
        target_bir_lowering=False,
        debug=False,
        enable_asserts=False,
        num_devices=NCORES,
    )

    WMAX = max(WT)
    # Processing order: start at the narrowest slot, then greedily take the
    # slot needing the fewest not-yet-loaded columns (windows overlap, so
    # increments are small); keep the widest slot off the last two positions.
    s0 = min(range(NT), key=lambda t: WT[t])
    order = [s0]
    lo, hi = OFFS[s0], OFFS[s0] + WT[s0]
    rest = set(range(NT)) - {s0}
    while rest:
        t = min(
            rest,
            key=lambda t: (
                max(0, lo - OFFS[t]) + max(0, OFFS[t] + WT[t] - hi),
                WT[t],
            ),
        )
        order.append(t)
        rest.remove(t)
        lo = min(lo, OFFS[t])
        hi = max(hi, OFFS[t] + WT[t])
    wmax_slot = max(range(NT), key=lambda t: WT[t])
    if order.index(wmax_slot) >= NT - 2 and NT >= 3:
        order.remove(wmax_slot)
        order.insert(NT - 3, wmax_slot)
    # The last slot's chain (edges -> matmul -> recip -> mul -> store) is
    # the kernel tail: prefer ending on the narrower of the final two.
    if NT >= 2 and WT[order[-2]] < WT[order[-1]]:
        order[-1], order[-2] = order[-2], order[-1]
    # Column-interval load chunks matching the final processing order.
    # The first slot's window is split into escalating pieces (256, 512,
    # ...) so its first edge op starts on a tiny head DMA and the rest
    # streams in behind it; each later slot flushes its left/right
    # extensions, merged with subsequent slots' claims on the same side up
    # to ~512 columns so tiny increments don't each pay a DMA.  Columns
    # outside every window are never read and never loaded.
    chunks = []
    H0 = (WT[s0] // 2 + 31) & ~31
    hsplits = [(0, H0), (H0, WT[s0])]
    for a, b in hsplits:
        chunks.append((OFFS[s0] + a, OFFS[s0] + b))
    lo, hi = OFFS[s0], OFFS[s0] + WT[s0]
    for i, t in enumerate(order[1:], 1):
        if OFFS[t] < lo:
            new_lo = OFFS[t]
            for u in order[i + 1 :]:
                cand = min(new_lo, OFFS[u])
                if lo - cand > max(512, lo - OFFS[t]):
                    break
                new_lo = cand
            chunks.append((new_lo, lo))
            lo = new_lo
        if OFFS[t] + WT[t] > hi:
            new_hi = OFFS[t] + WT[t]
            for u in order[i + 1 :]:
                cand = max(new_hi, OFFS[u] + WT[u])
                if cand - hi > max(512, OFFS[t] + WT[t] - hi):
                    break
                new_hi = cand
            chunks.append((hi, new_hi))
            hi = new_hi

    # DRAM I/O. boxes2 coord planes (x1,x2,y1,y2).  The first slot's
    # window (cb0) is host-replicated across all 128 partitions so the
    # pipeline head needs no on-chip processing; every other column is
    # host-replicated to only 32 partitions (cb, 4x less HBM traffic) and
    # expanded 32->64->128 on-chip by partition-shifted SBUF-to-SBUF
    # copies (step 1 on the GpSimd software-DGE queue, step 2 on the
    # hardware-DGE rings).
    W0 = WT[s0]
    O0 = OFFS[s0]
    cb0 = nc.dram_tensor("cb0", [P, 4, W0], f32, kind="ExternalInput").ap()
    cb = nc.dram_tensor("cb", [32, 4, WCOL], f32, kind="ExternalInput").ap()
    # Per-partition scalars: for i-tile t, columns t*5+k hold
    # (x1, x2, y1, y2, area1) of sorted boxes1 row t*128+p.
    sc = nc.dram_tensor("sc", [P, NT * 5], f32, kind="ExternalInput").ap()
    # Moving tensor for the union matmul: row 0 = a2e (boxes2 areas + eps),
    # row 1 = ones; with stationary [ones; a1] this gives a1_p + a2e_j.
    a2e2 = nc.dram_tensor("a2e2", [2, WCOL], f16, kind="ExternalInput").ap()
    # Stationary: row 0 = ones, row 1 = per-tile a1 (boxes1 areas).
    oa = nc.dram_tensor("oa", [2, NT * P], f16, kind="ExternalInput").ap()
    negi = nc.dram_tensor("negi", [P, P], f16, kind="ExternalInput").ap()
    out = nc.dram_tensor("out", [ROWS, WMAX], bf16, kind="ExternalOutput").ap()

    with tile.TileContext(nc) as tc, ExitStack() as ctx:
        bc = ctx.enter_context(tc.tile_pool(name="bc", bufs=1))
        scp = ctx.enter_context(tc.tile_pool(name="scp", bufs=1))
        work = ctx.enter_context(tc.tile_pool(name="work", bufs=4))
        outp = ctx.enter_context(tc.tile_pool(name="outp", bufs=3))
        psum = ctx.enter_context(tc.tile_pool(name="psum", bufs=1, space="PSUM"))

        sct = scp.tile([P, NT * 5], f32)
        ct = bc.tile([P, 4, WCOL], f32)
        a2e2t = scp.tile([2, WCOL], f16)
        oat = scp.tile([2, NT * P], f16)
        negit = scp.tile([P, P], f16)

        # Per-row scalars first (20KB; they gate the very first edge op),
        # then coordinate loads in processing-order chunks, one 2-D DMA per
        # coordinate plane, alternating between the two hardware-DGE rings
        # (Sync + Scalar).  x-planes (k=0,1) go first so the first rx edge
        # op can start as soon as they land.
        nc.sync.dma_start(sct[:], sc[:])
        for ci, (a, b) in enumerate(chunks):
            if ci < 2:
                # Head chunks: direct full-partition load from cb0.
                for k in range(4):
                    q = nc.sync if (ci * 4 + k) % 2 == 0 else nc.scalar
                    q.dma_start(ct[:, k, a:b], cb0[:, k, a - O0 : b - O0])
            else:
                for k in range(4):
                    q = nc.sync if (ci * 4 + k) % 2 == 0 else nc.scalar
                    q.dma_start(ct[0:32, k, a:b], cb[:, k, a:b])
                nc.gpsimd.dma_start(ct[32:64, :, a:b], ct[0:32, :, a:b])
                for k in range(4):
                    q = nc.scalar if (ci * 4 + k) % 2 == 0 else nc.sync
                    q.dma_start(ct[64:128, k, a:b], ct[0:64, k, a:b])
            if ci == 1:
                nc.sync.dma_start(negit[:], negi[:])
                nc.scalar.dma_start(oat[:], oa[:])
                nc.scalar.dma_start(a2e2t[:], a2e2[:])

        for ti, t in enumerate(order):
            o = OFFS[t]
            c = t * 5
            W = WT[t]
            rx = work.tile([P, WMAX], f16, tag="rx")
            ry = work.tile([P, WMAX], f16, tag="ry")
            inter = work.tile([P, WMAX], f16, tag="inter")
            rinv = work.tile([P, WMAX], f16, tag="rinv")
            ot = outp.tile([P, WMAX], bf16, tag="ot")

            # First tile: edge ops in escalating column pieces so each
            # starts as soon as its head DMA lands.  Last tile: split in
            # half so the tail chain (edges -> ... -> store) is shorter.
            NCH = -(-W // PS)
            h = ((-(-NCH // 2)) // 2) * 2 * PS
            if ti == 0:
                echunks = hsplits
            elif ti == NT - 1 and 0 < h < W:
                echunks = [(0, h), (h, W)]
            else:
                echunks = [(0, W)]
            for e0, e1 in echunks:
                nc.vector._custom_dve(
                    iou_edge,
                    out=rx[:, e0:e1],
                    in0=ct[:, 0, o + e0 : o + e1],
                    in1=ct[:, 1, o + e0 : o + e1],
                    s0=sct[:, c : c + 1],
                    s1=sct[:, c + 1 : c + 2],
                )
                nc.vector._custom_dve(
                    iou_edge,
                    out=ry[:, e0:e1],
                    in0=ct[:, 2, o + e0 : o + e1],
                    in1=ct[:, 3, o + e0 : o + e1],
                    s0=sct[:, c + 2 : c + 3],
                    s1=sct[:, c + 3 : c + 4],
                )
                nc.vector.tensor_mul(
                    inter[:, e0:e1], rx[:, e0:e1], ry[:, e0:e1]
                )

            # u = a1 + a2e - inter on TensorE: rank-2 [ones; a1] @
            # [[a2e],[ones]] plus (-I)@inter, per 512-col psum bank;
            # ScalarE Reciprocal over 2-bank chunks: rinv = 1/u.
            pts = []
            for k in range(0, NCH, 2):
                c0 = k * PS
                c1 = min(W, c0 + 2 * PS)
                pt = psum.tile([P, 2 * PS], f32, tag="pt", bufs=4)
                pt = pt[:, : c1 - c0]
                pts.append((pt, c0, c1))
                for m0 in range(0, c1 - c0, PS):
                    m1 = min(c1 - c0, m0 + PS)
                    nc.tensor.matmul(
                        pt[:, m0:m1],
                        oat[:, t * P : (t + 1) * P],
                        a2e2t[:, o + c0 + m0 : o + c0 + m1],
                        start=True,
                        stop=False,
                    )
            for pt, c0, c1 in pts:
                for m0 in range(0, c1 - c0, PS):
                    m1 = min(c1 - c0, m0 + PS)
                    nc.tensor.matmul(
                        pt[:, m0:m1],
                        negit[:],
                        inter[:, c0 + m0 : c0 + m1],
                        start=False,
                        stop=True,
                    )
            for pt, c0, c1 in pts:
                # ScalarE Reciprocal, emitted directly (the bass wrapper
                # refuses it wholesale; table accuracy is ~1e-3 relative,
                # well inside this kernel's 2e-2 budget — verified vs the
                # fp64 reference).
                nc.scalar.add_instruction(
                    mybir.InstActivation(
                        name=nc.get_next_instruction_name(),
                        func=act.Reciprocal,
                        ins=[
                            nc.scalar.lower_ap(pt[:]),
                            mybir.ImmediateValue(
                                dtype=mybir.dt.float32, value=0.0
                            ),
                            mybir.ImmediateValue(
                                dtype=mybir.dt.float32, value=1.0
                            ),
                            mybir.ImmediateValue(
                                dtype=mybir.dt.float32, value=0.0
                            ),
                        ],
                        outs=[nc.scalar.lower_ap(rinv[:, c0:c1])],
                    )
                )

            # Last tile: split the output stage and store via the (by now
            # idle) hardware-DGE rings, so the final store is short and
            # doesn't pay the software-DGE completion latency.
            if ti == NT - 1 and 0 < h < W:
                ochunks = [(0, h), (h, W)]
            else:
                ochunks = [(0, W)]
            for oi, (a, b) in enumerate(ochunks):
                nc.vector.tensor_mul(ot[:, a:b], inter[:, a:b], rinv[:, a:b])
                if ti == NT - 1:
                    outq = nc.sync if oi % 2 == 0 else nc.scalar
                else:
                    outq = nc.gpsimd
                outq.dma_start(out[t * P : (t + 1) * P, a:b], ot[:, a:b])

    nc.compile()
    return nc


def _get_program(WT, OFFS, WCOL):
    key = (tuple(WT), tuple(OFFS), WCOL)
    if key not in _COMPILED:
        _COMPILED[key] = _build_program(list(WT), list(OFFS), WCOL)
    return _COMPILED[key]


def _plan(boxes1, boxes2):
    """Two-level windowing: rows are x-sorted into per-core bands; within
    each core, rows are y-sorted and each core's x-relevant boxes2 subset
    is Y1-sorted, so each 128-row i-tile needs only a contiguous window of
    y-overlapping columns.  Per-slot offsets/widths (compile-time) are
    solved as an LP.  Returns those plus per-core packing data."""
    b1 = np.ascontiguousarray(boxes1, dtype=np.float32)
    b2 = np.ascontiguousarray(boxes2, dtype=np.float32)
    p1 = np.argsort(b1[:, 0], kind="stable")
    s1 = b1[p1]
    X1_2, Y1_2 = b2[:, 0], b2[:, 1]
    X2_2, Y2_2 = b2[:, 2], b2[:, 3]
    wmax2 = float((X2_2 - X1_2).max())
    hmax2 = float((Y2_2 - Y1_2).max())
    x1order = np.argsort(X1_2, kind="stable")
    X1s = X1_2[x1order]

    rowids = []  # per core: original boxes1 ids in slot order  [ROWS]
    colids = []  # per core: original boxes2 ids, Y1-sorted x-relevant subset
    jL = np.empty((NCORES, NT), np.int64)
    jR = np.empty((NCORES, NT), np.int64)
    for c in range(NCORES):
        blk = s1[c * ROWS : (c + 1) * ROWS]
        lo = np.searchsorted(X1s, np.float32(blk[:, 0].min() - wmax2) - 1e-3)
        hi = np.searchsorted(X1s, np.float32(blk[:, 2].max()) + 1e-3)
        cj = x1order[lo:hi]
        cj = cj[np.argsort(Y1_2[cj], kind="stable")]
        colids.append(cj)
        Y1c = Y1_2[cj]
        yord = np.argsort(blk[:, 1], kind="stable")
        blky = blk[yord]
        jl = np.empty(NT, np.int64)
        jr = np.empty(NT, np.int64)
        for t in range(NT):
            rows = blky[t * P : (t + 1) * P]
            jl[t] = np.searchsorted(
                Y1c, np.float32(rows[:, 1].min() - hmax2) - 1e-3
            )
            jr[t] = np.searchsorted(Y1c, np.float32(rows[:, 3].max()) + 1e-3)
        # Slot t = t-th y-ordered group: window positions then decompose
        # additively (base_c + off_t), which is what the LP below needs.
        jL[c] = jl
        jR[c] = jr
        rowids.append(p1[c * ROWS + yord])

    # Decompose window starts into base_c + off_t minimizing total width:
    # an LP over (off_t, base_c, W_t) with containment constraints.
    offs = None
    try:
        from scipy.optimize import linprog

        nv = 2 * NT + NCORES
        A_ub, b_ub = [], []
        for c in range(NCORES):
            for t in range(NT):
                r1 = np.zeros(nv)
                r1[t] = 1
                r1[NT + c] = 1
                A_ub.append(r1)
                b_ub.append(jL[c, t])
                r2 = np.zeros(nv)
                r2[t] = -1
                r2[NT + c] = -1
                r2[NT + NCORES + t] = -1
                A_ub.append(r2)
                b_ub.append(-jR[c, t])
        cvec = np.zeros(nv)
        cvec[NT + NCORES :] = 1
        res = linprog(
            cvec,
            A_ub=np.array(A_ub),
            b_ub=np.array(b_ub),
            bounds=[(None, None)] * (NT + NCORES) + [(0, None)] * NT,
            method="highs",
        )
        if res.status == 0:
            offs = np.floor(res.x[:NT]).astype(np.int64)
    except Exception:
        pass
    if offs is None:
        offs = np.median(jL - jL[:, :1], axis=0).astype(np.int64)
    offs = 2 * ((offs - offs.min()) // 2)
    bases = (jL - offs[None, :]).min(axis=1)
    wt = (jR - offs[None, :] - bases[:, None]).max(axis=0)
    WT = [min(int(-(-max(int(w), 64) // 32) * 32), M + 512) for w in wt]
    WCOL = int(max(offs[t] + WT[t] for t in range(NT)))
    return dict(
        b1=b1, b2=b2, rowids=rowids, colids=colids,
        WT=WT, OFFS=[int(o) for o in offs], WCOL=WCOL, bases=bases,
    )


def _make_in_maps(plan):
    b1, b2 = plan["b1"], plan["b2"]
    WCOL, bases = plan["WCOL"], plan["bases"]
    WT, OFFS = plan["WT"], plan["OFFS"]
    # Same choice as _build_program: the narrowest slot leads.
    s0 = min(range(NT), key=lambda t: WT[t])
    plan["O0"], plan["W0"] = OFFS[s0], WT[s0]

    a2e = (
        (b2[:, 2] - b2[:, 0]) * (b2[:, 3] - b2[:, 1]) + np.float32(EPS)
    ).astype(np.float32)

    in_maps = []
    for c in range(NCORES):
        cj = plan["colids"][c]
        idx = bases[c] + np.arange(WCOL)
        valid = (idx >= 0) & (idx < len(cj))
        idxc = cj[np.clip(idx, 0, len(cj) - 1)]
        pad = np.float32(-1e6)

        def rep(vec, fill):
            return np.where(valid, vec[idxc], fill).astype(np.float32)

        cbv = np.empty((4, WCOL), dtype=np.float32)
        cbv[0] = rep(b2[:, 0], pad)
        cbv[1] = rep(b2[:, 2], pad)
        cbv[2] = rep(b2[:, 1], pad)
        cbv[3] = rep(b2[:, 3], pad)
        o0, w0 = plan["O0"], plan["W0"]
        m = {
            "cb": np.ascontiguousarray(
                np.broadcast_to(cbv[None, :, :], (32, 4, WCOL))
            ),
            "cb0": np.ascontiguousarray(
                np.broadcast_to(cbv[None, :, o0 : o0 + w0], (P, 4, w0))
            ),
        }
        rows = b1[plan["rowids"][c]].reshape(NT, P, 4)
        a1 = (rows[:, :, 2] - rows[:, :, 0]) * (rows[:, :, 3] - rows[:, :, 1])
        scv = np.empty((P, NT * 5), dtype=np.float32)
        for t in range(NT):
            scv[:, t * 5 + 0] = rows[t, :, 0]
            scv[:, t * 5 + 1] = rows[t, :, 2]
            scv[:, t * 5 + 2] = rows[t, :, 1]
            scv[:, t * 5 + 3] = rows[t, :, 3]
            scv[:, t * 5 + 4] = a1[t]
        m["sc"] = scv
        a2e2 = np.ones((2, WCOL), np.float16)
        a2e2[0] = np.where(valid, a2e[idxc], np.float32(1.0)).astype(
            np.float16
        )
        m["a2e2"] = a2e2
        oa = np.ones((2, NT * P), np.float16)
        oa[1] = a1.reshape(-1).astype(np.float16)
        m["oa"] = oa
        m["negi"] = (-np.eye(P)).astype(np.float16)
        in_maps.append(m)
    return in_maps


def _assemble(plan, results):
    """Scatter per-tile blocks into the full fp32 matrix."""
    WT, OFFS, bases = plan["WT"], plan["OFFS"], plan["bases"]

    out = np.zeros((N, M), dtype=np.float32)
    for c in range(NCORES):
        blk = np.asarray(results[c]["out"])
        cj = plan["colids"][c]
        rids = plan["rowids"][c]
        for t in range(NT):
            c0 = bases[c] + OFFS[t]
            c1 = c0 + WT[t]
            s0 = max(0, -c0)
            cc0 = max(0, c0)
            cc1 = min(len(cj), c1)
            if cc1 <= cc0:
                continue
            vals = blk[t * P : (t + 1) * P, s0 : s0 + (cc1 - cc0)].astype(
                np.float32
            )
            out[rids[t * P : (t + 1) * P][:, None], cj[cc0:cc1][None, :]] = vals
    return out


def _run(inputs, trace=False, tmpdir=None):
    from concourse.bass_utils import run_bass_kernel_spmd

    plan = _plan(inputs["boxes1"], inputs["boxes2"])
    nc = _get_program(plan["WT"], plan["OFFS"], plan["WCOL"])
    in_maps = _make_in_maps(plan)
    kwargs = {}
    if trace:
        kwargs = dict(trace=True, tmpdir=tmpdir)
    res = run_bass_kernel_spmd(
        nc, in_maps, core_ids=list(range(NCORES)), **kwargs
    )
    return plan, res


def kernel(boxes1: np.ndarray, boxes2: np.ndarray) -> np.ndarray:
    plan, res = _run({"boxes1": boxes1, "boxes2": boxes2})
    return _assemble(plan, res.results)


# revision 45
# speedup vs baseline: 1.0332x; 1.0332x over previous
"""Pairwise IoU (8192x8192) on 8 Trainium2 NeuronCores via Bass/Tile.

Strategy
--------
Boxes can only overlap when both their x- and y-ranges are within the
max box extent of each other, so most of the 8192x8192 IoU matrix is
exactly zero.  Two-level windowing exploits this: boxes1 rows are
x-sorted and split across 8 cores (1024 rows each); within a core, rows
are y-sorted, and the core's x-relevant boxes2 subset (the only columns
that can ever overlap its rows) is Y1-sorted.  Each 128-row i-tile then
only scores a contiguous window of ~1.2-2.6k y-overlapping columns
(~18% of M on average).  Provably-zero entries are assembled on the
host.

The device program is SPMD (one program, 8 cores), so per-tile window
offsets/widths are compile-time constants OFFS[t]/WT[t]; the host packs
each core's columns so that its tile-t window lies inside
[base_c + OFFS[t], base_c + OFFS[t] + WT[t]).  Per-core tile groups are
assigned to slots in width-descending order (so the max-over-cores per
slot is tight), then the (base, OFFS, WT) decomposition is solved as an
LP from the actual data at first call.

Per-core device kernel, per [128, WT] output tile:
  rx    = relu(min(x2_i, X2_j) - max(x1_i, X1_j))   custom DVE op (fp16 out)
  ry    = same for y                                custom DVE op (fp16 out)
  inter = rx*ry                                     DVE tensor_tensor (fp16, 2x)
  u     = a1_i + a2e_j - inter                      TensorE (rank-2 [ones;a1]
                                                    + (-I)@inter, fp16) -> PSUM
  rinv  = Reciprocal(u)                             ScalarE LUT, one pass
  out   = inter * rinv                              DVE tensor_tensor (bf16 out)

Coordinate planes are host-replicated across partitions and streamed in
as per-plane 2-D DMAs over both hardware-DGE rings (Sync + Scalar) in
processing-order chunks; output stores ride the GpSimd software-DGE
queue so they never contend with input descriptor generation.  (An
on-chip partition-broadcast variant was measured slower: the GpSimd Q7
daisy chain floods the shared POOL SBUF port and inflates every
concurrent DVE op by 2-6x.)
"""

import numpy as np

N = 8192
M = 8192
NCORES = 8
ROWS = N // NCORES  # rows of boxes1 per core
P = 128  # partitions
NT = ROWS // P  # 8 i-tiles per core
PS = 512  # psum bank width (fp32)
EPS = 1e-7

_COMPILED = {}


def _register_op(name, spec, subdim=False):
    import concourse.dve_ops as dve_ops
    from concourse.dve_spec import lower
    from concourse.dve_uop import DveOpSpec

    for op in dve_ops.OPS:
        if op.name == name:
            return op
    shas = {}
    for ver in ("v3", "v4"):
        try:
            shas[ver] = DveOpSpec(
                name=name, opcode=0, uops=lower(spec, ver=ver)
            ).sha(ver)
        except Exception:
            pass
    op = dve_ops.DveOp(name, spec, subdim=subdim, uops_sha=shas)
    dve_ops.OPS.append(op)
    dve_ops.CUSTOM_DVE_SPECS[op.name] = op.spec
    dve_ops._SUB_OPCODE_FOR_NAME[op.name] = (
        dve_ops._CUSTOM_DVE_ROW_BASE + len(dve_ops.OPS) - 1
    )
    return op


def _ensure_ops():
    """Register the IOU_EDGE custom DVE op (idempotent)."""
    from concourse.dve_spec import C0, C1, Spec, Src0, Src1, maxx, minn, relu

    edge = _register_op(
        "IOU_EDGE",
        Spec(
            body=relu(minn(Src1, C1) - maxx(Src0, C0)),
            reference=lambda in0, in1, s0, s1, imm2: np.maximum(
                np.minimum(in1, s1) - np.maximum(in0, s0), 0.0
            ).astype(np.float32),
        ),
    )
    return edge


def _build_program(WT, OFFS, WCOL):
    from contextlib import ExitStack

    import concourse.bacc as bacc
    import concourse.mybir as mybir
    import concourse.tile as tile

    iou_edge = _ensure_ops()

    f32 = mybir.dt.float32
    f16 = mybir.dt.float16
    bf16 = mybir.dt.bfloat16
    act = mybir.ActivationFunctionType
    nc = bacc.Bacc(
        "TRN2",
        target_bir_lowering=False,
        debug=False,
        enable_asserts=False,
        num_devices=NCORES,
    )

    WMAX = max(WT)
    # Processing order: start at the narrowest slot, then greedily take the
    # slot needing the fewest not-yet-loaded columns (windows overlap, so
    # increments are small); keep the widest slot off the last two positions.
    s0 = min(range(NT), key=lambda t: WT[t])
    order = [s0]
    lo, hi = OFFS[s0], OFFS[s0] + WT[s0]
    rest = set(range(NT)) - {s0}
    while rest:
        t = min(
            rest,
            key=lambda t: (
                max(0, lo - OFFS[t]) + max(0, OFFS[t] + WT[t] - hi),
                WT[t],
            ),
        )
        order.append(t)
        rest.remove(t)
        lo = min(lo, OFFS[t])
        hi = max(hi, OFFS[t] + WT[t])
    wmax_slot = max(range(NT), key=lambda t: WT[t])
    if order.index(wmax_slot) >= NT - 2 and NT >= 3:
        order.remove(wmax_slot)
        order.insert(NT - 3, wmax_slot)
    # The last slot's chain (edges -> matmul -> recip -> mul -> store) is
    # the kernel tail: prefer ending on the narrower of the final two.
    if NT >= 2 and WT[order[-2]] < WT[order[-1]]:
        order[-1], order[-2] = order[-2], order[-1]
    # Column-interval load chunks matching the final processing order.
    # The first slot's window is split into escalating pieces (256, 512,
    # ...) so its first edge op starts on a tiny head DMA and the rest
    # streams in behind it; each later slot flushes its left/right
    # extensions, merged with subsequent slots' claims on the same side up
    # to ~512 columns so tiny increments don't each pay a DMA.  Columns
    # outside every window are never read and never loaded.
    chunks = []
    H0 = (WT[s0] // 2 + 31) & ~31
    hsplits = [(0, H0), (H0, WT[s0])]
    for a, b in hsplits:
        chunks.append((OFFS[s0] + a, OFFS[s0] + b))
    lo, hi = OFFS[s0], OFFS[s0] + WT[s0]
    for i, t in enumerate(order[1:], 1):
        if OFFS[t] < lo:
            new_lo = OFFS[t]
            for u in order[i + 1 :]:
                cand = min(new_lo, OFFS[u])
                if lo - cand > max(512, lo - OFFS[t]):
                    break
                new_lo = cand
            chunks.append((new_lo, lo))
            lo = new_lo
        if OFFS[t] + WT[t] > hi:
            new_hi = OFFS[t] + WT[t]
            for u in order[i + 1 :]:
                cand = max(new_hi, OFFS[u] + WT[u])
                if cand - hi > max(512, OFFS[t] + WT[t] - hi):
                    break
                new_hi = cand
            chunks.append((hi, new_hi))
            hi = new_hi

    # DRAM I/O. boxes2 coord planes (x1,x2,y1,y2).  The first slot's
    # window (cb0) is host-replicated across all 128 partitions so the
    # pipeline head needs no on-chip processing; every other column is
    # host-replicated to 64 partitions (cb, halving its HBM traffic) and
    # expanded 64->128 by one partition-shifted SBUF-to-SBUF copy on the
    # same hardware-DGE rings (rides the fabric, not HBM).
    W0 = WT[s0]
    O0 = OFFS[s0]
    cb0 = nc.dram_tensor("cb0", [P, 4, W0], f32, kind="ExternalInput").ap()
    cb = nc.dram_tensor("cb", [64, 4, WCOL], f32, kind="ExternalInput").ap()
    # Per-partition scalars: for i-tile t, columns t*5+k hold
    # (x1, x2, y1, y2, area1) of sorted boxes1 row t*128+p.
    sc = nc.dram_tensor("sc", [P, NT * 5], f32, kind="ExternalInput").ap()
    # Moving tensor for the union matmul: row 0 = a2e (boxes2 areas + eps),
    # row 1 = ones; with stationary [ones; a1] this gives a1_p + a2e_j.
    a2e2 = nc.dram_tensor("a2e2", [2, WCOL], f16, kind="ExternalInput").ap()
    # Stationary: row 0 = ones, row 1 = per-tile a1 (boxes1 areas).
    oa = nc.dram_tensor("oa", [2, NT * P], f16, kind="ExternalInput").ap()
    negi = nc.dram_tensor("negi", [P, P], f16, kind="ExternalInput").ap()
    out = nc.dram_tensor("out", [ROWS, WMAX], bf16, kind="ExternalOutput").ap()

    with tile.TileContext(nc) as tc, ExitStack() as ctx:
        bc = ctx.enter_context(tc.tile_pool(name="bc", bufs=1))
        scp = ctx.enter_context(tc.tile_pool(name="scp", bufs=1))
        work = ctx.enter_context(tc.tile_pool(name="work", bufs=4))
        outp = ctx.enter_context(tc.tile_pool(name="outp", bufs=3))
        psum = ctx.enter_context(tc.tile_pool(name="psum", bufs=1, space="PSUM"))

        sct = scp.tile([P, NT * 5], f32)
        ct = bc.tile([P, 4, WCOL], f32)
        a2e2t = scp.tile([2, WCOL], f16)
        oat = scp.tile([2, NT * P], f16)
        negit = scp.tile([P, P], f16)

        # Per-row scalars first (20KB; they gate the very first edge op),
        # then coordinate loads in processing-order chunks, one 2-D DMA per
        # coordinate plane, alternating between the two hardware-DGE rings
        # (Sync + Scalar).  x-planes (k=0,1) go first so the first rx edge
        # op can start as soon as they land.
        nc.sync.dma_start(sct[:], sc[:])
        for ci, (a, b) in enumerate(chunks):
            if ci < 2:
                # Head chunks: direct full-partition load from cb0.
                for k in range(4):
                    q = nc.sync if (ci * 4 + k) % 2 == 0 else nc.scalar
                    q.dma_start(ct[:, k, a:b], cb0[:, k, a - O0 : b - O0])
            else:
                for k in range(4):
                    q = nc.sync if (ci * 4 + k) % 2 == 0 else nc.scalar
                    q.dma_start(ct[0:64, k, a:b], cb[:, k, a:b])
                for k in range(4):
                    q = nc.scalar if (ci * 4 + k) % 2 == 0 else nc.sync
                    q.dma_start(ct[64:128, k, a:b], ct[0:64, k, a:b])
            if ci == 1:
                nc.sync.dma_start(negit[:], negi[:])
                nc.scalar.dma_start(oat[:], oa[:])
                nc.scalar.dma_start(a2e2t[:], a2e2[:])

        for ti, t in enumerate(order):
            o = OFFS[t]
            c = t * 5
            W = WT[t]
            rx = work.tile([P, WMAX], f16, tag="rx")
            ry = work.tile([P, WMAX], f16, tag="ry")
            inter = work.tile([P, WMAX], f16, tag="inter")
            rinv = work.tile([P, WMAX], f16, tag="rinv")
            ot = outp.tile([P, WMAX], bf16, tag="ot")

            # First tile: edge ops in escalating column pieces so each
            # starts as soon as its head DMA lands.  Last tile: split in
            # half so the tail chain (edges -> ... -> store) is shorter.
            NCH = -(-W // PS)
            h = ((-(-NCH // 2)) // 2) * 2 * PS
            if ti == 0:
                echunks = hsplits
            elif ti == NT - 1 and 0 < h < W:
                echunks = [(0, h), (h, W)]
            else:
                echunks = [(0, W)]
            for e0, e1 in echunks:
                nc.vector._custom_dve(
                    iou_edge,
                    out=rx[:, e0:e1],
                    in0=ct[:, 0, o + e0 : o + e1],
                    in1=ct[:, 1, o + e0 : o + e1],
                    s0=sct[:, c : c + 1],
                    s1=sct[:, c + 1 : c + 2],
                )
                nc.vector._custom_dve(
                    iou_edge,
                    out=ry[:, e0:e1],
                    in0=ct[:, 2, o + e0 : o + e1],
                    in1=ct[:, 3, o + e0 : o + e1],
                    s0=sct[:, c + 2 : c + 3],
                    s1=sct[:, c + 3 : c + 4],
                )
                nc.vector.tensor_mul(
                    inter[:, e0:e1], rx[:, e0:e1], ry[:, e0:e1]
                )

            # u = a1 + a2e - inter on TensorE: rank-2 [ones; a1] @
            # [[a2e],[ones]] plus (-I)@inter, per 512-col psum bank;
            # ScalarE Reciprocal over 2-bank chunks: rinv = 1/u.
            pts = []
            for k in range(0, NCH, 2):
                c0 = k * PS
                c1 = min(W, c0 + 2 * PS)
                pt = psum.tile([P, 2 * PS], f32, tag="pt", bufs=4)
                pt = pt[:, : c1 - c0]
                pts.append((pt, c0, c1))
                for m0 in range(0, c1 - c0, PS):
                    m1 = min(c1 - c0, m0 + PS)
                    nc.tensor.matmul(
                        pt[:, m0:m1],
                        oat[:, t * P : (t + 1) * P],
                        a2e2t[:, o + c0 + m0 : o + c0 + m1],
                        start=True,
                        stop=False,
                    )
            for pt, c0, c1 in pts:
                for m0 in range(0, c1 - c0, PS):
                    m1 = min(c1 - c0, m0 + PS)
                    nc.tensor.matmul(
                        pt[:, m0:m1],
                        negit[:],
                        inter[:, c0 + m0 : c0 + m1],
                        start=False,
                        stop=True,
                    )
            for pt, c0, c1 in pts:
                # ScalarE Reciprocal, emitted directly (the bass wrapper
                # refuses it wholesale; table accuracy is ~1e-3 relative,
                # well inside this kernel's 2e-2 budget — verified vs the
                # fp64 reference).
                nc.scalar.add_instruction(
                    mybir.InstActivation(
                        name=nc.get_next_instruction_name(),
                        func=act.Reciprocal,
                        ins=[
                            nc.scalar.lower_ap(pt[:]),
                            mybir.ImmediateValue(
                                dtype=mybir.dt.float32, value=0.0
                            ),
                            mybir.ImmediateValue(
                                dtype=mybir.dt.float32, value=1.0
                            ),
                            mybir.ImmediateValue(
                                dtype=mybir.dt.float32, value=0.0
                            ),
                        ],
                        outs=[nc.scalar.lower_ap(rinv[:, c0:c1])],
                    )
                )

            # Last tile: split the output stage and store via the (by now
            # idle) hardware-DGE rings, so the final store is short and
            # doesn't pay the software-DGE completion latency.
            if ti == NT - 1 and 0 < h < W:
                ochunks = [(0, h), (h, W)]
            else:
                ochunks = [(0, W)]
            for oi, (a, b) in enumerate(ochunks):
                nc.vector.tensor_mul(ot[:, a:b], inter[:, a:b], rinv[:, a:b])
                if ti == NT - 1:
                    outq = nc.sync if oi % 2 == 0 else nc.scalar
                else:
                    outq = nc.gpsimd
                outq.dma_start(out[t * P : (t + 1) * P, a:b], ot[:, a:b])

    nc.compile()
    return nc


def _get_program(WT, OFFS, WCOL):
    key = (tuple(WT), tuple(OFFS), WCOL)
    if key not in _COMPILED:
        _COMPILED[key] = _build_program(list(WT), list(OFFS), WCOL)
    return _COMPILED[key]


def _plan(boxes1, boxes2):
    """Two-level windowing: rows are x-sorted into per-core bands; within
    each core, rows are y-sorted and each core's x-relevant boxes2 subset
    is Y1-sorted, so each 128-row i-tile needs only a contiguous window of
    y-overlapping columns.  Per-slot offsets/widths (compile-time) are
    solved as an LP.  Returns those plus per-core packing data."""
    b1 = np.ascontiguousarray(boxes1, dtype=np.float32)
    b2 = np.ascontiguousarray(boxes2, dtype=np.float32)
    p1 = np.argsort(b1[:, 0], kind="stable")
    s1 = b1[p1]
    X1_2, Y1_2 = b2[:, 0], b2[:, 1]
    X2_2, Y2_2 = b2[:, 2], b2[:, 3]
    wmax2 = float((X2_2 - X1_2).max())
    hmax2 = float((Y2_2 - Y1_2).max())
    x1order = np.argsort(X1_2, kind="stable")
    X1s = X1_2[x1order]

    rowids = []  # per core: original boxes1 ids in slot order  [ROWS]
    colids = []  # per core: original boxes2 ids, Y1-sorted x-relevant subset
    jL = np.empty((NCORES, NT), np.int64)
    jR = np.empty((NCORES, NT), np.int64)
    for c in range(NCORES):
        blk = s1[c * ROWS : (c + 1) * ROWS]
        lo = np.searchsorted(X1s, np.float32(blk[:, 0].min() - wmax2) - 1e-3)
        hi = np.searchsorted(X1s, np.float32(blk[:, 2].max()) + 1e-3)
        cj = x1order[lo:hi]
        cj = cj[np.argsort(Y1_2[cj], kind="stable")]
        colids.append(cj)
        Y1c = Y1_2[cj]
        yord = np.argsort(blk[:, 1], kind="stable")
        blky = blk[yord]
        jl = np.empty(NT, np.int64)
        jr = np.empty(NT, np.int64)
        for t in range(NT):
            rows = blky[t * P : (t + 1) * P]
            jl[t] = np.searchsorted(
                Y1c, np.float32(rows[:, 1].min() - hmax2) - 1e-3
            )
            jr[t] = np.searchsorted(Y1c, np.float32(rows[:, 3].max()) + 1e-3)
        # Slot t = t-th y-ordered group: window positions then decompose
        # additively (base_c + off_t), which is what the LP below needs.
        jL[c] = jl
        jR[c] = jr
        rowids.append(p1[c * ROWS + yord])

    # Decompose window starts into base_c + off_t minimizing total width:
    # an LP over (off_t, base_c, W_t) with containment constraints.
    offs = None
    try:
        from scipy.optimize import linprog

        nv = 2 * NT + NCORES
        A_ub, b_ub = [], []
        for c in range(NCORES):
            for t in range(NT):
                r1 = np.zeros(nv)
                r1[t] = 1
                r1[NT + c] = 1
                A_ub.append(r1)
                b_ub.append(jL[c, t])
                r2 = np.zeros(nv)
                r2[t] = -1
                r2[NT + c] = -1
                r2[NT + NCORES + t] = -1
                A_ub.append(r2)
                b_ub.append(-jR[c, t])
        cvec = np.zeros(nv)
        cvec[NT + NCORES :] = 1
        res = linprog(
            cvec,
            A_ub=np.array(A_ub),
            b_ub=np.array(b_ub),
            bounds=[(None, None)] * (NT + NCORES) + [(0, None)] * NT,
            method="highs",
        )
        if res.status == 0:
            offs = np.floor(res.x[:NT]).astype(np.int64)
    except Exception:
        pass
    if offs is None:
        offs = np.median(jL - jL[:, :1], axis=0).astype(np.int64)
    offs = 2 * ((offs - offs.min()) // 2)
    bases = (jL - offs[None, :]).min(axis=1)
    wt = (jR - offs[None, :] - bases[:, None]).max(axis=0)
    WT = [min(int(-(-max(int(w), 64) // 32) * 32), M + 512) for w in wt]
    WCOL = int(max(offs[t] + WT[t] for t in range(NT)))
    return dict(
        b1=b1, b2=b2, rowids=rowids, colids=colids,
        WT=WT, OFFS=[int(o) for o in offs], WCOL=WCOL, bases=bases,
    )


def _make_in_maps(plan):
    b1, b2 = plan["b1"], plan["b2"]
    WCOL, bases = plan["WCOL"], plan["bases"]
    WT, OFFS = plan["WT"], plan["OFFS"]
    # Same choice as _build_program: the narrowest slot leads.
    s0 = min(range(NT), key=lambda t: WT[t])
    plan["O0"], plan["W0"] = OFFS[s0], WT[s0]

    a2e = (
        (b2[:, 2] - b2[:, 0]) * (b2[:, 3] - b2[:, 1]) + np.float32(EPS)
    ).astype(np.float32)

    in_maps = []
    for c in range(NCORES):
        cj = plan["colids"][c]
        idx = bases[c] + np.arange(WCOL)
        valid = (idx >= 0) & (idx < len(cj))
        idxc = cj[np.clip(idx, 0, len(cj) - 1)]
        pad = np.float32(-1e6)

        def rep(vec, fill):
            return np.where(valid, vec[idxc], fill).astype(np.float32)

        cbv = np.empty((4, WCOL), dtype=np.float32)
        cbv[0] = rep(b2[:, 0], pad)
        cbv[1] = rep(b2[:, 2], pad)
        cbv[2] = rep(b2[:, 1], pad)
        cbv[3] = rep(b2[:, 3], pad)
        o0, w0 = plan["O0"], plan["W0"]
        m = {
            "cb": np.ascontiguousarray(
                np.broadcast_to(cbv[None, :, :], (64, 4, WCOL))
            ),
            "cb0": np.ascontiguousarray(
                np.broadcast_to(cbv[None, :, o0 : o0 + w0], (P, 4, w0))
            ),
        }
        rows = b1[plan["rowids"][c]].reshape(NT, P, 4)
        a1 = (rows[:, :, 2] - rows[:, :, 0]) * (rows[:, :, 3] - rows[:, :, 1])
        scv = np.empty((P, NT * 5), dtype=np.float32)
        for t in range(NT):
            scv[:, t * 5 + 0] = rows[t, :, 0]
            scv[:, t * 5 + 1] = rows[t, :, 2]
            scv[:, t * 5 + 2] = rows[t, :, 1]
            scv[:, t * 5 + 3] = rows[t, :, 3]
            scv[:, t * 5 + 4] = a1[t]
        m["sc"] = scv
        a2e2 = np.ones((2, WCOL), np.float16)
        a2e2[0] = np.where(valid, a2e[idxc], np.float32(1.0)).astype(
            np.float16
        )
        m["a2e2"] = a2e2
        oa = np.ones((2, NT * P), np.float16)
        oa[1] = a1.reshape(-1).astype(np.float16)
        m["oa"] = oa
        m["negi"] = (-np.eye(P)).astype(np.float16)
        in_maps.append(m)
    return in_maps


def _assemble(plan, results):
    """Scatter per-tile blocks into the full fp32 matrix."""
    WT, OFFS, bases = plan["WT"], plan["OFFS"], plan["bases"]

    out = np.zeros((N, M), dtype=np.float32)
    for c in range(NCORES):
        blk = np.asarray(results[c]["out"])
        cj = plan["colids"][c]
        rids = plan["rowids"][c]
        for t in range(NT):
            c0 = bases[c] + OFFS[t]
            c1 = c0 + WT[t]
            s0 = max(0, -c0)
            cc0 = max(0, c0)
            cc1 = min(len(cj), c1)
            if cc1 <= cc0:
                continue
            vals = blk[t * P : (t + 1) * P, s0 : s0 + (cc1 - cc0)].astype(
                np.float32
            )
            out[rids[t * P : (t + 1) * P][:, None], cj[cc0:cc1][None, :]] = vals
    return out


def _run(inputs, trace=False, tmpdir=None):
    from concourse.bass_utils import run_bass_kernel_spmd

    plan = _plan(inputs["boxes1"], inputs["boxes2"])
    nc = _get_program(plan["WT"], plan["OFFS"], plan["WCOL"])
    in_maps = _make_in_maps(plan)
    kwargs = {}
    if trace:
        kwargs = dict(trace=True, tmpdir=tmpdir)
    res = run_bass_kernel_spmd(
        nc, in_maps, core_ids=list(range(NCORES)), **kwargs
    )
    return plan, res


def kernel(boxes1: np.ndarray, boxes2: np.ndarray) -> np.ndarray:
    plan, res = _run({"boxes1": boxes1, "boxes2": boxes2})
    return _assemble(plan, res.results)


# revision 49
# speedup vs baseline: 1.1315x; 1.0952x over previous
"""Pairwise IoU (8192x8192) on 8 Trainium2 NeuronCores via Bass/Tile.

Strategy
--------
Boxes can only overlap when both their x- and y-ranges are within the
max box extent of each other, so most of the 8192x8192 IoU matrix is
exactly zero.  Two-level windowing exploits this: boxes1 rows are
x-sorted and split across 8 cores (1024 rows each); within a core, rows
are y-sorted, and the core's x-relevant boxes2 subset (the only columns
that can ever overlap its rows) is Y1-sorted.  Each 128-row i-tile then
only scores a contiguous window of ~1.2-2.6k y-overlapping columns
(~18% of M on average).  Provably-zero entries are assembled on the
host.

The device program is SPMD (one program, 8 cores), so per-tile window
offsets/widths are compile-time constants OFFS[t]/WT[t]; the host packs
each core's columns so that its tile-t window lies inside
[base_c + OFFS[t], base_c + OFFS[t] + WT[t]).  Per-core tile groups are
assigned to slots in width-descending order (so the max-over-cores per
slot is tight), then the (base, OFFS, WT) decomposition is solved as an
LP from the actual data at first call.

Per-core device kernel, per [128, WT] output tile:
  rx    = relu(min(x2_i, X2_j) - max(x1_i, X1_j))   custom DVE op (fp16 out)
  ry    = same for y                                custom DVE op (fp16 out)
  inter = rx*ry                                     DVE tensor_tensor (fp16, 2x)
  u     = a1_i + a2e_j - inter                      TensorE (rank-2 [ones;a1]
                                                    + (-I)@inter, fp16) -> PSUM
  rinv  = Reciprocal(u)                             ScalarE LUT, one pass
  out   = inter * rinv                              DVE tensor_tensor (bf16 out)

Coordinate planes are host-replicated across partitions and streamed in
as per-plane 2-D DMAs over both hardware-DGE rings (Sync + Scalar) in
processing-order chunks; output stores ride the GpSimd software-DGE
queue so they never contend with input descriptor generation.  (An
on-chip partition-broadcast variant was measured slower: the GpSimd Q7
daisy chain floods the shared POOL SBUF port and inflates every
concurrent DVE op by 2-6x.)
"""

import numpy as np

N = 8192
M = 8192
NCORES = 8
ROWS = N // NCORES  # rows of boxes1 per core
P = 128  # partitions
NT = ROWS // P  # 8 i-tiles per core
PS = 512  # psum bank width (fp32)
EPS = 1e-7

_COMPILED = {}


def _register_op(name, spec, subdim=False):
    import concourse.dve_ops as dve_ops
    from concourse.dve_spec import lower
    from concourse.dve_uop import DveOpSpec

    for op in dve_ops.OPS:
        if op.name == name:
            return op
    shas = {}
    for ver in ("v3", "v4"):
        try:
            shas[ver] = DveOpSpec(
                name=name, opcode=0, uops=lower(spec, ver=ver)
            ).sha(ver)
        except Exception:
            pass
    op = dve_ops.DveOp(name, spec, subdim=subdim, uops_sha=shas)
    dve_ops.OPS.append(op)
    dve_ops.CUSTOM_DVE_SPECS[op.name] = op.spec
    dve_ops._SUB_OPCODE_FOR_NAME[op.name] = (
        dve_ops._CUSTOM_DVE_ROW_BASE + len(dve_ops.OPS) - 1
    )
    return op


def _ensure_ops():
    """Register the IOU_EDGE custom DVE op (idempotent)."""
    from concourse.dve_spec import C0, C1, Spec, Src0, Src1, maxx, minn, relu

    edge = _register_op(
        "IOU_EDGE",
        Spec(
            body=relu(minn(Src1, C1) - maxx(Src0, C0)),
            reference=lambda in0, in1, s0, s1, imm2: np.maximum(
                np.minimum(in1, s1) - np.maximum(in0, s0), 0.0
            ).astype(np.float32),
        ),
    )
    return edge


def _build_program(WT, OFFS, WCOL):
    from contextlib import ExitStack

    import concourse.bacc as bacc
    import concourse.mybir as mybir
    import concourse.tile as tile

    iou_edge = _ensure_ops()

    f32 = mybir.dt.float32
    f16 = mybir.dt.float16
    bf16 = mybir.dt.bfloat16
    act = mybir.ActivationFunctionType
    nc = bacc.Bacc(
        "TRN2",
        target_bir_lowering=False,
        debug=False,
        enable_asserts=False,
        num_devices=NCORES,
    )

    WMAX = max(WT)
    # Processing order: start at the narrowest slot, then greedily take the
    # slot needing the fewest not-yet-loaded columns (windows overlap, so
    # increments are small); keep the widest slot off the last two positions.
    s0 = min(range(NT), key=lambda t: WT[t])
    order = [s0]
    lo, hi = OFFS[s0], OFFS[s0] + WT[s0]
    rest = set(range(NT)) - {s0}
    while rest:
        t = min(
            rest,
            key=lambda t: (
                max(0, lo - OFFS[t]) + max(0, OFFS[t] + WT[t] - hi),
                WT[t],
            ),
        )
        order.append(t)
        rest.remove(t)
        lo = min(lo, OFFS[t])
        hi = max(hi, OFFS[t] + WT[t])
    wmax_slot = max(range(NT), key=lambda t: WT[t])
    if order.index(wmax_slot) >= NT - 2 and NT >= 3:
        order.remove(wmax_slot)
        order.insert(NT - 3, wmax_slot)
    # The last slot's chain (edges -> matmul -> recip -> mul -> store) is
    # the kernel tail: prefer ending on the narrower of the final two.
    if NT >= 2 and WT[order[-2]] < WT[order[-1]]:
        order[-1], order[-2] = order[-2], order[-1]
    # Column-interval load chunks matching the final processing order.
    # The first slot's window is split into escalating pieces (256, 512,
    # ...) so its first edge op starts on a tiny head DMA and the rest
    # streams in behind it; each later slot flushes its left/right
    # extensions, merged with subsequent slots' claims on the same side up
    # to ~512 columns so tiny increments don't each pay a DMA.  Columns
    # outside every window are never read and never loaded.
    chunks = []
    H0 = (WT[s0] // 2 + 31) & ~31
    hsplits = [(0, H0), (H0, WT[s0])]
    for a, b in hsplits:
        chunks.append((OFFS[s0] + a, OFFS[s0] + b))
    lo, hi = OFFS[s0], OFFS[s0] + WT[s0]
    for i, t in enumerate(order[1:], 1):
        if OFFS[t] < lo:
            new_lo = OFFS[t]
            for u in order[i + 1 :]:
                cand = min(new_lo, OFFS[u])
                if lo - cand > max(512, lo - OFFS[t]):
                    break
                new_lo = cand
            chunks.append((new_lo, lo))
            lo = new_lo
        if OFFS[t] + WT[t] > hi:
            new_hi = OFFS[t] + WT[t]
            for u in order[i + 1 :]:
                cand = max(new_hi, OFFS[u] + WT[u])
                if cand - hi > max(512, OFFS[t] + WT[t] - hi):
                    break
                new_hi = cand
            chunks.append((hi, new_hi))
            hi = new_hi

    # DRAM I/O. boxes2 coord planes (x1,x2,y1,y2), host-replicated across
    # the 128 partitions.  (Partial replication with on-chip partition
    # expansion was measured slower in every variant: the Q7 broadcast op
    # floods the shared POOL SBUF port, and SBUF-to-SBUF doubling copies
    # serialize behind loads in the DGE ring FIFOs.)
    cb = nc.dram_tensor("cb", [P, 4, WCOL], f32, kind="ExternalInput").ap()
    # Per-partition scalars: for i-tile t, columns t*5+k hold
    # (x1, x2, y1, y2, area1) of sorted boxes1 row t*128+p.
    sc = nc.dram_tensor("sc", [P, NT * 5], f32, kind="ExternalInput").ap()
    # Moving tensor for the union matmul: row 0 = a2e (boxes2 areas + eps),
    # row 1 = ones; with stationary [ones; a1] this gives a1_p + a2e_j.
    a2e2 = nc.dram_tensor("a2e2", [2, WCOL], f16, kind="ExternalInput").ap()
    # Stationary: row 0 = ones, row 1 = per-tile a1 (boxes1 areas).
    oa = nc.dram_tensor("oa", [2, NT * P], f16, kind="ExternalInput").ap()
    negi = nc.dram_tensor("negi", [P, P], f16, kind="ExternalInput").ap()
    out = nc.dram_tensor("out", [ROWS, WMAX], bf16, kind="ExternalOutput").ap()

    with tile.TileContext(nc) as tc, ExitStack() as ctx:
        bc = ctx.enter_context(tc.tile_pool(name="bc", bufs=1))
        scp = ctx.enter_context(tc.tile_pool(name="scp", bufs=1))
        work = ctx.enter_context(tc.tile_pool(name="work", bufs=4))
        outp = ctx.enter_context(tc.tile_pool(name="outp", bufs=3))
        psum = ctx.enter_context(tc.tile_pool(name="psum", bufs=1, space="PSUM"))

        sct = scp.tile([P, NT * 5], f32)
        ct = bc.tile([P, 4, WCOL], f32)
        a2e2t = scp.tile([2, WCOL], f16)
        oat = scp.tile([2, NT * P], f16)
        negit = scp.tile([P, P], f16)

        # Per-row scalars first (20KB; they gate the very first edge op),
        # then coordinate loads in processing-order chunks, one 2-D DMA per
        # coordinate plane, alternating between the two hardware-DGE rings
        # (Sync + Scalar).  x-planes (k=0,1) go first so the first rx edge
        # op can start as soon as they land.
        nc.sync.dma_start(sct[:], sc[:])
        for ci, (a, b) in enumerate(chunks):
            for k in range(4):
                q = nc.sync if (ci * 4 + k) % 2 == 0 else nc.scalar
                q.dma_start(ct[:, k, a:b], cb[:, k, a:b])
            if ci == 1:
                nc.sync.dma_start(negit[:], negi[:])
                nc.scalar.dma_start(oat[:], oa[:])
                nc.scalar.dma_start(a2e2t[:], a2e2[:])

        for ti, t in enumerate(order):
            o = OFFS[t]
            c = t * 5
            W = WT[t]
            rx = work.tile([P, WMAX], f16, tag="rx")
            ry = work.tile([P, WMAX], f16, tag="ry")
            inter = work.tile([P, WMAX], f16, tag="inter")
            rinv = work.tile([P, WMAX], f16, tag="rinv")
            ot = outp.tile([P, WMAX], bf16, tag="ot")

            # First tile: edge ops in escalating column pieces so each
            # starts as soon as its head DMA lands.  Last tile: split in
            # half so the tail chain (edges -> ... -> store) is shorter.
            NCH = -(-W // PS)
            h = ((-(-NCH // 2)) // 2) * 2 * PS
            if ti == 0:
                echunks = hsplits
            elif ti == NT - 1 and 0 < h < W:
                echunks = [(0, h), (h, W)]
            else:
                echunks = [(0, W)]
            for e0, e1 in echunks:
                nc.vector._custom_dve(
                    iou_edge,
                    out=rx[:, e0:e1],
                    in0=ct[:, 0, o + e0 : o + e1],
                    in1=ct[:, 1, o + e0 : o + e1],
                    s0=sct[:, c : c + 1],
                    s1=sct[:, c + 1 : c + 2],
                )
                nc.vector._custom_dve(
                    iou_edge,
                    out=ry[:, e0:e1],
                    in0=ct[:, 2, o + e0 : o + e1],
                    in1=ct[:, 3, o + e0 : o + e1],
                    s0=sct[:, c + 2 : c + 3],
                    s1=sct[:, c + 3 : c + 4],
                )
                nc.vector.tensor_mul(
                    inter[:, e0:e1], rx[:, e0:e1], ry[:, e0:e1]
                )

            # u = a1 + a2e - inter on TensorE: rank-2 [ones; a1] @
            # [[a2e],[ones]] plus (-I)@inter, per 512-col psum bank;
            # ScalarE Reciprocal over 2-bank chunks: rinv = 1/u.
            pts = []
            for k in range(0, NCH, 2):
                c0 = k * PS
                c1 = min(W, c0 + 2 * PS)
                pt = psum.tile([P, 2 * PS], f32, tag="pt", bufs=4)
                pt = pt[:, : c1 - c0]
                pts.append((pt, c0, c1))
                for m0 in range(0, c1 - c0, PS):
                    m1 = min(c1 - c0, m0 + PS)
                    nc.tensor.matmul(
                        pt[:, m0:m1],
                        oat[:, t * P : (t + 1) * P],
                        a2e2t[:, o + c0 + m0 : o + c0 + m1],
                        start=True,
                        stop=False,
                    )
            for pt, c0, c1 in pts:
                for m0 in range(0, c1 - c0, PS):
                    m1 = min(c1 - c0, m0 + PS)
                    nc.tensor.matmul(
                        pt[:, m0:m1],
                        negit[:],
                        inter[:, c0 + m0 : c0 + m1],
                        start=False,
                        stop=True,
                    )
            for pt, c0, c1 in pts:
                # ScalarE Reciprocal, emitted directly (the bass wrapper
                # refuses it wholesale; table accuracy is ~1e-3 relative,
                # well inside this kernel's 2e-2 budget — verified vs the
                # fp64 reference).
                nc.scalar.add_instruction(
                    mybir.InstActivation(
                        name=nc.get_next_instruction_name(),
                        func=act.Reciprocal,
                        ins=[
                            nc.scalar.lower_ap(pt[:]),
                            mybir.ImmediateValue(
                                dtype=mybir.dt.float32, value=0.0
                            ),
                            mybir.ImmediateValue(
                                dtype=mybir.dt.float32, value=1.0
                            ),
                            mybir.ImmediateValue(
                                dtype=mybir.dt.float32, value=0.0
                            ),
                        ],
                        outs=[nc.scalar.lower_ap(rinv[:, c0:c1])],
                    )
                )

            # Last tile: split the output stage and store via the (by now
            # idle) hardware-DGE rings, so the final store is short and
            # doesn't pay the software-DGE completion latency.
            if ti == NT - 1 and 0 < h < W:
                ochunks = [(0, h), (h, W)]
            else:
                ochunks = [(0, W)]
            for oi, (a, b) in enumerate(ochunks):
                nc.vector.tensor_mul(ot[:, a:b], inter[:, a:b], rinv[:, a:b])
                if ti == NT - 1:
                    outq = nc.sync if oi % 2 == 0 else nc.scalar
                else:
                    outq = nc.gpsimd
                outq.dma_start(out[t * P : (t + 1) * P, a:b], ot[:, a:b])

    nc.compile()
    return nc


def _get_program(WT, OFFS, WCOL):
    key = (tuple(WT), tuple(OFFS), WCOL)
    if key not in _COMPILED:
        _COMPILED[key] = _build_program(list(WT), list(OFFS), WCOL)
    return _COMPILED[key]


def _plan(boxes1, boxes2):
    """Two-level windowing: rows are x-sorted into per-core bands; within
    each core, rows are y-sorted and each core's x-relevant boxes2 subset
    is Y1-sorted, so each 128-row i-tile needs only a contiguous window of
    y-overlapping columns.  Per-slot offsets/widths (compile-time) are
    solved as an LP.  Returns those plus per-core packing data."""
    b1 = np.ascontiguousarray(boxes1, dtype=np.float32)
    b2 = np.ascontiguousarray(boxes2, dtype=np.float32)
    p1 = np.argsort(b1[:, 0], kind="stable")
    s1 = b1[p1]
    X1_2, Y1_2 = b2[:, 0], b2[:, 1]
    X2_2, Y2_2 = b2[:, 2], b2[:, 3]
    wmax2 = float((X2_2 - X1_2).max())
    hmax2 = float((Y2_2 - Y1_2).max())
    x1order = np.argsort(X1_2, kind="stable")
    X1s = X1_2[x1order]

    rowids = []  # per core: original boxes1 ids in slot order  [ROWS]
    colids = []  # per core: original boxes2 ids, Y1-sorted x-relevant subset
    jL = np.empty((NCORES, NT), np.int64)
    jR = np.empty((NCORES, NT), np.int64)
    for c in range(NCORES):
        blk = s1[c * ROWS : (c + 1) * ROWS]
        lo = np.searchsorted(X1s, np.float32(blk[:, 0].min() - wmax2) - 1e-3)
        hi = np.searchsorted(X1s, np.float32(blk[:, 2].max()) + 1e-3)
        cj = x1order[lo:hi]
        cj = cj[np.argsort(Y1_2[cj], kind="stable")]
        colids.append(cj)
        Y1c = Y1_2[cj]
        yord = np.argsort(blk[:, 1], kind="stable")
        blky = blk[yord]
        jl = np.empty(NT, np.int64)
        jr = np.empty(NT, np.int64)
        for t in range(NT):
            rows = blky[t * P : (t + 1) * P]
            jl[t] = np.searchsorted(
                Y1c, np.float32(rows[:, 1].min() - hmax2) - 1e-3
            )
            jr[t] = np.searchsorted(Y1c, np.float32(rows[:, 3].max()) + 1e-3)
        # Slot t = t-th y-ordered group: window positions then decompose
        # additively (base_c + off_t), which is what the LP below needs.
        jL[c] = jl
        jR[c] = jr
        rowids.append(p1[c * ROWS + yord])

    # Decompose window starts into base_c + off_t minimizing total width:
    # an LP over (off_t, base_c, W_t) with containment constraints.
    offs = None
    try:
        from scipy.optimize import linprog

        nv = 2 * NT + NCORES
        A_ub, b_ub = [], []
        for c in range(NCORES):
            for t in range(NT):
                r1 = np.zeros(nv)
                r1[t] = 1
                r1[NT + c] = 1
                A_ub.append(r1)
                b_ub.append(jL[c, t])
                r2 = np.zeros(nv)
                r2[t] = -1
                r2[NT + c] = -1
                r2[NT + NCORES + t] = -1
                A_ub.append(r2)
                b_ub.append(-jR[c, t])
        cvec = np.zeros(nv)
        cvec[NT + NCORES :] = 1
        res = linprog(
            cvec,
            A_ub=np.array(A_ub),
            b_ub=np.array(b_ub),
            bounds=[(None, None)] * (NT + NCORES) + [(0, None)] * NT,
            method="highs",
        )
        if res.status == 0:
            offs = np.floor(res.x[:NT]).astype(np.int64)
    except Exception:
        pass
    if offs is None:
        offs = np.median(jL - jL[:, :1], axis=0).astype(np.int64)
    offs = 2 * ((offs - offs.min()) // 2)
    bases = (jL - offs[None, :]).min(axis=1)
    wt = (jR - offs[None, :] - bases[:, None]).max(axis=0)
    WT = [min(int(-(-max(int(w), 64) // 32) * 32), M + 512) for w in wt]
    WCOL = int(max(offs[t] + WT[t] for t in range(NT)))
    return dict(
        b1=b1, b2=b2, rowids=rowids, colids=colids,
        WT=WT, OFFS=[int(o) for o in offs], WCOL=WCOL, bases=bases,
    )


def _make_in_maps(plan):
    b1, b2 = plan["b1"], plan["b2"]
    WCOL, bases = plan["WCOL"], plan["bases"]


    a2e = (
        (b2[:, 2] - b2[:, 0]) * (b2[:, 3] - b2[:, 1]) + np.float32(EPS)
    ).astype(np.float32)

    in_maps = []
    for c in range(NCORES):
        cj = plan["colids"][c]
        idx = bases[c] + np.arange(WCOL)
        valid = (idx >= 0) & (idx < len(cj))
        idxc = cj[np.clip(idx, 0, len(cj) - 1)]
        pad = np.float32(-1e6)

        def rep(vec, fill):
            return np.where(valid, vec[idxc], fill).astype(np.float32)

        cbv = np.empty((4, WCOL), dtype=np.float32)
        cbv[0] = rep(b2[:, 0], pad)
        cbv[1] = rep(b2[:, 2], pad)
        cbv[2] = rep(b2[:, 1], pad)
        cbv[3] = rep(b2[:, 3], pad)
        m = {
            "cb": np.ascontiguousarray(
                np.broadcast_to(cbv[None, :, :], (P, 4, WCOL))
            )
        }
        rows = b1[plan["rowids"][c]].reshape(NT, P, 4)
        a1 = (rows[:, :, 2] - rows[:, :, 0]) * (rows[:, :, 3] - rows[:, :, 1])
        scv = np.empty((P, NT * 5), dtype=np.float32)
        for t in range(NT):
            scv[:, t * 5 + 0] = rows[t, :, 0]
            scv[:, t * 5 + 1] = rows[t, :, 2]
            scv[:, t * 5 + 2] = rows[t, :, 1]
            scv[:, t * 5 + 3] = rows[t, :, 3]
            scv[:, t * 5 + 4] = a1[t]
        m["sc"] = scv
        a2e2 = np.ones((2, WCOL), np.float16)
        a2e2[0] = np.where(valid, a2e[idxc], np.float32(1.0)).astype(
            np.float16
        )
        m["a2e2"] = a2e2
        oa = np.ones((2, NT * P), np.float16)
        oa[1] = a1.reshape(-1).astype(np.float16)
        m["oa"] = oa
        m["negi"] = (-np.eye(P)).astype(np.float16)
        in_maps.append(m)
    return in_maps


def _assemble(plan, results):
    """Scatter per-tile blocks into the full fp32 matrix."""
    WT, OFFS, bases = plan["WT"], plan["OFFS"], plan["bases"]

    out = np.zeros((N, M), dtype=np.float32)
    for c in range(NCORES):
        blk = np.asarray(results[c]["out"])
        cj = plan["colids"][c]
        rids = plan["rowids"][c]
        for t in range(NT):
            c0 = bases[c] + OFFS[t]
            c1 = c0 + WT[t]
            s0 = max(0, -c0)
            cc0 = max(0, c0)
            cc1 = min(len(cj), c1)
            if cc1 <= cc0:
                continue
            vals = blk[t * P : (t + 1) * P, s0 : s0 + (cc1 - cc0)].astype(
                np.float32
            )
            out[rids[t * P : (t + 1) * P][:, None], cj[cc0:cc1][None, :]] = vals
    return out


def _run(inputs, trace=False, tmpdir=None):
    from concourse.bass_utils import run_bass_kernel_spmd

    plan = _plan(inputs["boxes1"], inputs["boxes2"])
    nc = _get_program(plan["WT"], plan["OFFS"], plan["WCOL"])
    in_maps = _make_in_maps(plan)
    kwargs = {}
    if trace:
        kwargs = dict(trace=True, tmpdir=tmpdir)
    res = run_bass_kernel_spmd(
        nc, in_maps, core_ids=list(range(NCORES)), **kwargs
    )
    return plan, res


def kernel(boxes1: np.ndarray, boxes2: np.ndarray) -> np.ndarray:
    plan, res = _run({"boxes1": boxes1, "boxes2": boxes2})
    return _assemble(plan, res.results)
